# revision 35
# baseline (speedup 1.0000x reference)
"""DTNN layer kernel for Trainium2 (8 NeuronCores).

Math: out[b,i,o] = sum_j sum_h Wfc[o,h] * hx[b,i,h] * hd[b,i,j,h]
with hx = x@Wcf.T + bcf, hd = dist@Wdf.T + bdf.
Since Wfc/Wdf are linear, the j-sum commutes:
    ds[b,i,d]  = sum_j dist[b,i,j,d]                  (memory-bound reduction)
    out[b,i,:] = ((x@Wcf.T + bcf) * (ds@Wdf.T + N*bdf)) @ Wfc.T
So the kernel streams `distance` once (134MB) and does a few 128x128 matmuls.

Sharding: flatten (B,N) -> 1024 i-rows, 128 rows per core; no cross-core comms.

Measured (NTFF profile, core 0): ~70us/core, vs ~47us pure HBM stream at the
358 GB/s per-core fair share plus ~13us fixed NEFF prologue/epilogue and a
~9us serial tail. Structure:
- dist is streamed as a few big HWDGE DMAs on one ring (in-order arrivals);
  DVE folds each tile to 128 columns in place right after it lands (halving
  unit-stride adds run at full DVE rate; strided reduces were 1.6x slower).
- biases are folded into PE matmuls as K=1 rank-1 updates, and the
  (hx * N*bdf) @ WfcT bias term is preloaded into the output PSUM during the
  stream so the post-stream tail is just transpose -> Wdf matmul -> mul ->
  accumulate-matmul -> store.
"""

import numpy as np

import concourse.bass as bass
import concourse.bacc as bacc
import concourse.mybir as mybir
from concourse.tile import TileContext
from concourse.bass_utils import run_bass_kernel_spmd

B, N, D, H = 4, 256, 128, 128
NCORES = 8
ROWS = B * N // NCORES  # 128 i-rows per core
FP = mybir.dt.float32

# packed constant columns: [xT | wcfT | wdfT | wfcT | eye | rows...]
C_XT = 0
C_WCF = 128
C_WDF = 256
C_WFC = 384
C_EYE = 512
C_BCFR = 640   # partition 0: bcf row (1, H)
C_BDFR = 768   # partition 0: bdf row (1, H)
C_ONES = 896   # partition 0: ones row (1, ROWS)
C_BDFC = 1024  # bdf as a per-partition column (H, 1)
C_TOT = 1025


def build_nc():
    nc = bacc.Bacc("TRN2", target_bir_lowering=False)
    dist = nc.declare_dram_parameter("dist", [ROWS, N * D], FP, isOutput=False)
    cst = nc.declare_dram_parameter("cst", [128, C_TOT], FP, isOutput=False)
    out = nc.declare_dram_parameter("out", [ROWS, D], FP, isOutput=True)

    with TileContext(nc) as tc:
        with (
            tc.tile_pool(name="const", bufs=1) as cpool,
            tc.tile_pool(name="dist", bufs=1) as dpool,
            tc.tile_pool(name="work", bufs=1) as wpool,
            tc.tile_pool(name="psum", bufs=1, space="PSUM") as ppool,
        ):
            # Issue the dist stream first so the big DMAs start ASAP; the
            # constants ride behind them on the same queue.
            SIZES = [64, 64, 64, 32, 16, 8, 4, 4]  # j-counts per DMA tile
            dtiles = []
            off = 0
            for k, jn in enumerate(SIZES):
                t = dpool.tile([ROWS, jn * D], FP, tag=f"dist{k}")
                # Single HWDGE ring (SP): in-order arrivals matching the DVE
                # fold order; the stream is HBM-fair-share-bound (~358GB/s)
                # so a second ring adds no bandwidth, only ordering jitter.
                nc.sync.dma_start(out=t[:], in_=dist[:, off * D:(off + jn) * D])
                dtiles.append(t)
                off += jn

            cst_t = cpool.tile([128, C_TOT], FP)
            nc.scalar.dma_start(out=cst_t[:], in_=cst[:])
            xT_t = cst_t[:, C_XT:C_XT + ROWS]
            wcf_t = cst_t[:, C_WCF:C_WCF + H]
            wdf_t = cst_t[:, C_WDF:C_WDF + H]
            wfc_t = cst_t[:, C_WFC:C_WFC + D]
            ident = cst_t[:, C_EYE:C_EYE + ROWS]
            bcf_row = cst_t[0:1, C_BCFR:C_BCFR + H]
            ones_row = cst_t[0:1, C_ONES:C_ONES + ROWS]

            # hx^T = (Wcf^T)^T @ x^T + bcf x ones -> (H, ROWS) in PSUM
            hx_ps = ppool.tile([H, ROWS], FP)
            nc.tensor.matmul(hx_ps[:], wcf_t, xT_t, start=True, stop=False)
            nc.tensor.matmul(hx_ps[:], bcf_row, ones_row, start=False, stop=True)
            hxT = wpool.tile([H, ROWS], FP)
            nc.vector.tensor_copy(hxT[:], hx_ps[:])

            # Preload the bias term (hx * N*bdf) @ Wfc^T into the output
            # PSUM during the stream; the tail's out-matmul accumulates
            # onto it, removing the bias matmul from the critical tail.
            bdfN = wpool.tile([H, 1], FP)
            nc.vector.tensor_scalar_mul(bdfN[:], cst_t[:, C_BDFC:C_BDFC + 1],
                                        float(N))
            s0T = wpool.tile([H, ROWS], FP)
            nc.vector.tensor_scalar_mul(s0T[:], hxT[:], bdfN[:])
            out_ps = ppool.tile([ROWS, D], FP)
            nc.tensor.matmul(out_ps[:], s0T[:], wfc_t, start=True, stop=False)

            # Streaming j-reduction: ds[i,d] = sum_j dist[i,j,d].
            # Each tile is folded to 128 columns in place immediately after
            # its DMA lands (halving adds, all unit-stride = full DVE rate),
            # then added into the running accumulator (tile 0). Per-tile DVE
            # work (~4.9us) keeps pace with per-tile DMA arrival (~5.1us),
            # so only ~2us of DVE work remains after the last (half-size)
            # tile arrives.
            # Per-tile: fold to 128 cols (unit-stride halving adds at full
            # DVE rate), PE-transpose the partial ds_k, copy to SBUF, and
            # accumulate Wdf^T.T @ dsT_k into hd_ps. Transpose and the Wdf
            # matmul are linear, so partials sum in PSUM — everything except
            # the last tile's short chain hides under the DMA stream.
            hd_ps = ppool.tile([H, ROWS], FP)
            last = len(SIZES) - 1
            for k, jn in enumerate(SIZES):
                t = dtiles[k]
                half = jn * D // 2
                while half >= D:
                    nc.vector.tensor_add(
                        t[:, 0:half], t[:, 0:half], t[:, half:2 * half]
                    )
                    half //= 2
                dsT_ps = ppool.tile([D, ROWS], FP, tag="dsT_ps")
                nc.tensor.transpose(dsT_ps[:], t[:, 0:D], ident)
                dsT = wpool.tile([D, ROWS], FP, tag=f"dsT{k}")
                nc.vector.tensor_copy(dsT[:], dsT_ps[:])
                nc.tensor.matmul(hd_ps[:], wdf_t, dsT[:],
                                 start=(k == 0), stop=(k == last),
                                 skip_group_check=True)

            # s^T = hx^T * hd^T (one PSUM operand max per DVE op)
            sT = wpool.tile([H, ROWS], FP)
            nc.vector.tensor_mul(sT[:], hd_ps[:], hxT[:])

            # out += sT^T @ Wfc^T, accumulating onto the preloaded bias term
            nc.tensor.matmul(out_ps[:], sT[:], wfc_t, start=False, stop=True,
                             skip_group_check=True)
            out_sb = wpool.tile([ROWS, D], FP)
            nc.vector.tensor_copy(out_sb[:], out_ps[:])
            nc.sync.dma_start(out=out[:], in_=out_sb[:])
    nc.compile()
    return nc


_NC_CACHE = None


def _get_nc():
    global _NC_CACHE
    if _NC_CACHE is None:
        _NC_CACHE = build_nc()
    return _NC_CACHE


def _make_in_maps(x, distance, Wcf_w, Wcf_b, Wdf_w, Wdf_b, Wfc_w):
    x = np.ascontiguousarray(np.asarray(x, np.float32))
    distance = np.ascontiguousarray(np.asarray(distance, np.float32))
    x_flat = x.reshape(B * N, D)
    dist_flat = distance.reshape(B * N, N * D)
    wcfT = np.asarray(Wcf_w, np.float32).T
    wdfT = np.asarray(Wdf_w, np.float32).T
    wfcT = np.asarray(Wfc_w, np.float32).T
    bcf = np.asarray(Wcf_b, np.float32)
    bdf = np.asarray(Wdf_b, np.float32)
    in_maps = []
    for c in range(NCORES):
        sl = slice(c * ROWS, (c + 1) * ROWS)
        cstblk = np.zeros((128, C_TOT), np.float32)
        cstblk[:, C_XT:C_XT + ROWS] = x_flat[sl].T
        cstblk[:, C_WCF:C_WCF + H] = wcfT
        cstblk[:, C_WDF:C_WDF + H] = wdfT
        cstblk[:, C_WFC:C_WFC + D] = wfcT
        cstblk[:, C_EYE:C_EYE + ROWS] = np.eye(ROWS, dtype=np.float32)
        cstblk[0, C_BCFR:C_BCFR + H] = bcf
        cstblk[0, C_BDFR:C_BDFR + H] = bdf
        cstblk[0, C_ONES:C_ONES + ROWS] = 1.0
        cstblk[:, C_BDFC] = bdf
        in_maps.append({
            "dist": np.ascontiguousarray(dist_flat[sl]),
            "cst": cstblk,
        })
    return in_maps


def kernel(x, distance, Wcf_w, Wcf_b, Wdf_w, Wdf_b, Wfc_w):
    in_maps = _make_in_maps(x, distance, Wcf_w, Wcf_b, Wdf_w, Wdf_b, Wfc_w)
    nc = _get_nc()
    res = run_bass_kernel_spmd(nc, in_maps, list(range(NCORES))).results
    out = np.concatenate([res[c]["out"] for c in range(NCORES)], axis=0)
    return out.reshape(B, N, D)


# revision 36
# speedup vs baseline: 1.0561x; 1.0561x over previous
"""DTNN layer kernel for Trainium2 (8 NeuronCores).

Math: out[b,i,o] = sum_j sum_h Wfc[o,h] * hx[b,i,h] * hd[b,i,j,h]
with hx = x@Wcf.T + bcf, hd = dist@Wdf.T + bdf.
Since Wfc/Wdf are linear, the j-sum commutes:
    ds[b,i,d]  = sum_j dist[b,i,j,d]                  (memory-bound reduction)
    out[b,i,:] = ((x@Wcf.T + bcf) * (ds@Wdf.T + N*bdf)) @ Wfc.T
So the kernel streams `distance` once (134MB) and does a few 128x128 matmuls.

Sharding: flatten (B,N) -> 1024 i-rows, 128 rows per core; no cross-core comms.

Measured (NTFF profile, core 0): ~70us/core, vs ~47us pure HBM stream at the
358 GB/s per-core fair share plus ~13us fixed NEFF prologue/epilogue and a
~9us serial tail. Structure:
- dist is streamed as a few big HWDGE DMAs on one ring (in-order arrivals);
  DVE folds each tile to 128 columns in place right after it lands (halving
  unit-stride adds run at full DVE rate; strided reduces were 1.6x slower).
- biases are folded into PE matmuls as K=1 rank-1 updates, and the
  (hx * N*bdf) @ WfcT bias term is preloaded into the output PSUM during the
  stream so the post-stream tail is just transpose -> Wdf matmul -> mul ->
  accumulate-matmul -> store.
"""

import numpy as np

import concourse.bass as bass
import concourse.bacc as bacc
import concourse.mybir as mybir
from concourse.tile import TileContext
from concourse.bass_utils import run_bass_kernel_spmd

B, N, D, H = 4, 256, 128, 128
NCORES = 8
ROWS = B * N // NCORES  # 128 i-rows per core
FP = mybir.dt.float32

# packed constant columns: [xT | wcfT | wdfT | wfcT | eye | rows...]
C_XT = 0
C_WCF = 128
C_WDF = 256
C_WFC = 384
C_EYE = 512
C_BCFR = 640   # partition 0: bcf row (1, H)
C_BDFR = 768   # partition 0: bdf row (1, H)
C_ONES = 896   # partition 0: ones row (1, ROWS)
C_BDFC = 1024  # bdf as a per-partition column (H, 1)
C_TOT = 1025


def build_nc():
    nc = bacc.Bacc("TRN2", target_bir_lowering=False)
    dist = nc.declare_dram_parameter("dist", [ROWS, N * D], FP, isOutput=False)
    cst = nc.declare_dram_parameter("cst", [128, C_TOT], FP, isOutput=False)
    out = nc.declare_dram_parameter("out", [ROWS, D], FP, isOutput=True)

    with TileContext(nc) as tc:
        with (
            tc.tile_pool(name="const", bufs=1) as cpool,
            tc.tile_pool(name="dist", bufs=1) as dpool,
            tc.tile_pool(name="work", bufs=1) as wpool,
            tc.tile_pool(name="psum", bufs=1, space="PSUM") as ppool,
        ):
            # Issue the dist stream first so the big DMAs start ASAP; the
            # constants ride behind them on the same queue.
            SIZES = [64, 64, 64, 32, 16, 8, 4, 4]  # j-counts per DMA tile
            dtiles = []
            off = 0
            for k, jn in enumerate(SIZES):
                t = dpool.tile([ROWS, jn * D], FP, tag=f"dist{k}")
                # Single HWDGE ring (SP): in-order arrivals matching the DVE
                # fold order; the stream is HBM-fair-share-bound (~358GB/s)
                # so a second ring adds no bandwidth, only ordering jitter.
                nc.sync.dma_start(out=t[:], in_=dist[:, off * D:(off + jn) * D])
                dtiles.append(t)
                off += jn

            cst_t = cpool.tile([128, C_TOT], FP)
            nc.scalar.dma_start(out=cst_t[:], in_=cst[:])
            xT_t = cst_t[:, C_XT:C_XT + ROWS]
            wcf_t = cst_t[:, C_WCF:C_WCF + H]
            wdf_t = cst_t[:, C_WDF:C_WDF + H]
            wfc_t = cst_t[:, C_WFC:C_WFC + D]
            ident = cst_t[:, C_EYE:C_EYE + ROWS]
            bcf_row = cst_t[0:1, C_BCFR:C_BCFR + H]
            ones_row = cst_t[0:1, C_ONES:C_ONES + ROWS]

            # hx^T = (Wcf^T)^T @ x^T + bcf x ones -> (H, ROWS) in PSUM
            hx_ps = ppool.tile([H, ROWS], FP)
            nc.tensor.matmul(hx_ps[:], wcf_t, xT_t, start=True, stop=False)
            nc.tensor.matmul(hx_ps[:], bcf_row, ones_row, start=False, stop=True)
            hxT = wpool.tile([H, ROWS], FP)
            nc.vector.tensor_copy(hxT[:], hx_ps[:])

            # Preload the bias term (hx * N*bdf) @ Wfc^T into the output
            # PSUM during the stream; the tail's out-matmul accumulates
            # onto it, removing the bias matmul from the critical tail.
            bdfN = wpool.tile([H, 1], FP)
            nc.vector.tensor_scalar_mul(bdfN[:], cst_t[:, C_BDFC:C_BDFC + 1],
                                        float(N))
            s0T = wpool.tile([H, ROWS], FP)
            nc.vector.tensor_scalar_mul(s0T[:], hxT[:], bdfN[:])
            out_ps = ppool.tile([ROWS, D], FP)
            nc.tensor.matmul(out_ps[:], s0T[:], wfc_t, start=True, stop=False)

            # Streaming j-reduction: ds[i,d] = sum_j dist[i,j,d].
            # Each tile is folded to 128 columns in place immediately after
            # its DMA lands (halving adds, all unit-stride = full DVE rate),
            # then added into the running accumulator (tile 0). Per-tile DVE
            # work (~4.9us) keeps pace with per-tile DMA arrival (~5.1us),
            # so only ~2us of DVE work remains after the last (half-size)
            # tile arrives.
            acc = dtiles[0]
            for k, jn in enumerate(SIZES):
                t = dtiles[k]
                half = jn * D // 2
                while half >= D:
                    nc.vector.tensor_add(
                        t[:, 0:half], t[:, 0:half], t[:, half:2 * half]
                    )
                    half //= 2
                if k > 0:
                    nc.vector.tensor_add(acc[:, 0:D], acc[:, 0:D], t[:, 0:D])
            ds = acc[:, 0:D]

            # ds (i,d) -> dsT (d,i) via PE transpose
            dsT_ps = ppool.tile([D, ROWS], FP)
            nc.tensor.transpose(dsT_ps[:], ds, ident)
            dsT = wpool.tile([D, ROWS], FP)
            nc.vector.tensor_copy(dsT[:], dsT_ps[:])

            # hd^T (bias-free) = (Wdf^T)^T @ ds^T -> (H, ROWS)
            hd_ps = ppool.tile([H, ROWS], FP)
            nc.tensor.matmul(hd_ps[:], wdf_t, dsT[:], start=True, stop=True)

            # s^T = hx^T * hd^T (one PSUM operand max per DVE op)
            sT = wpool.tile([H, ROWS], FP)
            nc.vector.tensor_mul(sT[:], hd_ps[:], hxT[:])

            # out += sT^T @ Wfc^T, accumulating onto the preloaded bias term
            nc.tensor.matmul(out_ps[:], sT[:], wfc_t, start=False, stop=True,
                             skip_group_check=True)
            out_sb = wpool.tile([ROWS, D], FP)
            nc.vector.tensor_copy(out_sb[:], out_ps[:])
            nc.sync.dma_start(out=out[:], in_=out_sb[:])
    nc.compile()
    return nc


_NC_CACHE = None


def _get_nc():
    global _NC_CACHE
    if _NC_CACHE is None:
        _NC_CACHE = build_nc()
    return _NC_CACHE


def _make_in_maps(x, distance, Wcf_w, Wcf_b, Wdf_w, Wdf_b, Wfc_w):
    x = np.ascontiguousarray(np.asarray(x, np.float32))
    distance = np.ascontiguousarray(np.asarray(distance, np.float32))
    x_flat = x.reshape(B * N, D)
    dist_flat = distance.reshape(B * N, N * D)
    wcfT = np.asarray(Wcf_w, np.float32).T
    wdfT = np.asarray(Wdf_w, np.float32).T
    wfcT = np.asarray(Wfc_w, np.float32).T
    bcf = np.asarray(Wcf_b, np.float32)
    bdf = np.asarray(Wdf_b, np.float32)
    in_maps = []
    for c in range(NCORES):
        sl = slice(c * ROWS, (c + 1) * ROWS)
        cstblk = np.zeros((128, C_TOT), np.float32)
        cstblk[:, C_XT:C_XT + ROWS] = x_flat[sl].T
        cstblk[:, C_WCF:C_WCF + H] = wcfT
        cstblk[:, C_WDF:C_WDF + H] = wdfT
        cstblk[:, C_WFC:C_WFC + D] = wfcT
        cstblk[:, C_EYE:C_EYE + ROWS] = np.eye(ROWS, dtype=np.float32)
        cstblk[0, C_BCFR:C_BCFR + H] = bcf
        cstblk[0, C_BDFR:C_BDFR + H] = bdf
        cstblk[0, C_ONES:C_ONES + ROWS] = 1.0
        cstblk[:, C_BDFC] = bdf
        in_maps.append({
            "dist": np.ascontiguousarray(dist_flat[sl]),
            "cst": cstblk,
        })
    return in_maps


def kernel(x, distance, Wcf_w, Wcf_b, Wdf_w, Wdf_b, Wfc_w):
    in_maps = _make_in_maps(x, distance, Wcf_w, Wcf_b, Wdf_w, Wdf_b, Wfc_w)
    nc = _get_nc()
    res = run_bass_kernel_spmd(nc, in_maps, list(range(NCORES))).results
    out = np.concatenate([res[c]["out"] for c in range(NCORES)], axis=0)
    return out.reshape(B, N, D)
